# revision 75
# baseline (speedup 1.0000x reference)
"""AttentionPooling (segment softmax-pool) Trainium2 kernel, 8 NeuronCores.

Sharding: each core owns 32 consecutive segments (contiguous node range since
`batch` is sorted); the 32 segments split into G groups of spg segments whose
node ranges are padded to a fixed Gmax so all cores/groups run one static
graph.  All segment reductions are core-local; no collectives.

Per-core, per-group pipeline (matmuls bf16 in / fp32 accum):
  pass 1 (feature-major): scoresT = tanh(W1^T @ x^T + b1)^T @ W2 + b2
  middle: one-hot segment sums (DVE mult+reduce + PE cross-partition sum),
          softmax weights per the reference's scatter-add "max" stabilizer
          and the 1e-8 epsilon
  pass 2 (node-major):   out[seg, :] = (S^T * att)^T @ x   via PE accumulation

x is uploaded twice (node-major and feature-major, both bf16, partition-major
packed so every DMA reads >=4KB contiguous per partition); no on-chip
transposes.  Groups pipeline: group g's serial middle hides under group g+1's
pass-1 matmuls.
"""

import functools
import sys

sys.path.insert(0, "/opt/trn_rl_repo")

import ml_dtypes
import numpy as np

import concourse.bass as bass
import concourse.tile as tile
from concourse import bacc, mybir
from concourse.bass_utils import run_bass_kernel_spmd

NCORES = 8
NSEG = 256
HID = 512
H2 = 256  # hidden//2
SEG_PER_CORE = NSEG // NCORES  # 32

BF16 = mybir.dt.bfloat16
F32 = mybir.dt.float32
NPBF16 = ml_dtypes.bfloat16

G = 4  # groups per core
C_BATCH = 4  # pass-2 node chunks per DMA

DEBUG_TAPS = False


def _round_up(v, m):
    return (v + m - 1) // m * m


@functools.lru_cache(maxsize=4)
def _build_graph(g_groups, gmax, spg, b2val):
    C = gmax // 128  # node chunks per group
    F = gmax // 512  # pass-1 free-dim tiles per group

    nc = bacc.Bacc(None, target_bir_lowering=False, debug=False)
    # partition-major packed: every DMA reads contiguous bytes per partition
    x_nm = nc.declare_dram_parameter("x_nm", [g_groups, 128, C, HID], BF16, isOutput=False)
    x_fm = nc.declare_dram_parameter("x_fm", [g_groups, 128, F, 4, 512], BF16, isOutput=False)
    st_d = nc.declare_dram_parameter("st", [g_groups, 128, C, spg], F32, isOutput=False)
    w1_d = nc.declare_dram_parameter("w1", [HID, H2], BF16, isOutput=False)
    b1_d = nc.declare_dram_parameter("b1", [H2], F32, isOutput=False)
    w2_d = nc.declare_dram_parameter("w2", [H2, 1], BF16, isOutput=False)
    out_d = nc.declare_dram_parameter("out", [g_groups * spg, HID], F32, isOutput=True)
    seg_scratch = nc.dram_tensor("seg_scratch", [g_groups, 2, spg], F32)
    sc_scratch = nc.dram_tensor("sc_scratch", [g_groups, gmax], F32)
    dbg_d = None
    if DEBUG_TAPS:
        dbg_d = nc.declare_dram_parameter(
            "dbg", [g_groups, 4, 128, C], F32, isOutput=True
        )

    Tanh = mybir.ActivationFunctionType.Tanh
    Exp = mybir.ActivationFunctionType.Exp
    Copy = mybir.ActivationFunctionType.Copy

    with tile.TileContext(nc) as tc:
        with (
            tc.tile_pool(name="consts", bufs=1) as consts,
            tc.tile_pool(name="p1", bufs=13) as p1,
            tc.tile_pool(name="mid", bufs=3) as mid,
            tc.tile_pool(name="stp", bufs=4) as stp,
            tc.tile_pool(name="p2", bufs=8) as p2,
            tc.tile_pool(name="outp2", bufs=2) as outp2,
            tc.tile_pool(name="psum", bufs=3, space="PSUM") as psum,
            tc.tile_pool(name="psum_scp", bufs=2, space="PSUM") as psum_scp,
            tc.tile_pool(name="psum_small", bufs=1, space="PSUM") as psum_small,
            tc.tile_pool(name="psum_out", bufs=2, space="PSUM") as psum_out,
        ):
            # constants
            w1_sb = consts.tile([128, 4, H2], BF16)  # [p, kchunk, j]
            nc.sync.dma_start(out=w1_sb, in_=w1_d[:].rearrange("(k p) j -> p k j", p=128))
            b1_sb = consts.tile([128, 2], F32)  # [p, jchunk]
            nc.sync.dma_start(out=b1_sb, in_=b1_d[:].rearrange("(j p) -> p j", p=128))
            w2_sb = consts.tile([128, 2], BF16)
            nc.sync.dma_start(out=w2_sb, in_=w2_d[:].rearrange("(j p) o -> p (j o)", p=128))
            ones_sb = consts.tile([128, 1], F32)
            nc.vector.memset(ones_sb, 1.0)
            ones_row = consts.tile([1, 128], F32)
            nc.vector.memset(ones_row, 1.0)

            # engine pre-touches: keep later compute instructions at <=1
            # foreign wait and off the event-semaphore slow path.
            dum_act = consts.tile([128, 1], F32)
            nc.scalar.activation(dum_act, b1_sb[:, 0:1], Copy)
            dum_ps = psum_small.tile([128, 1], F32, tag="segred")
            nc.tensor.matmul(dum_ps, lhsT=w1_sb[:, 0, 0:128], rhs=w1_sb[:, 0, 0:1],
                             start=True, stop=True)
            dum_ps2 = psum_small.tile([1, 1], F32, tag="segred")
            nc.tensor.matmul(dum_ps2, lhsT=w2_sb[:, 0:1], rhs=w2_sb[:, 0:1],
                             start=True, stop=True)

            def emit_scores(g, f, tts):
                """second MLP layer + score write-out for tile f of group g;
                emitted one iteration late so PE never stalls on the tanh."""
                scp = psum_scp.tile([1, 512], F32, tag="scp")
                for j in range(2):
                    nc.tensor.matmul(
                        scp,
                        lhsT=w2_sb[:, j : j + 1],
                        rhs=tts[j],
                        start=(j == 0),
                        stop=(j == 1),
                    )
                srow = p1.tile([1, 512], F32, tag="srow")
                nc.vector.tensor_scalar_add(srow, scp, b2val)
                nc.gpsimd.dma_start(
                    out=sc_scratch[g, f * 512 : (f + 1) * 512][None, :],
                    in_=srow[:, :],
                )

            # Per-group state for the software-pipelined schedule.
            S = [dict() for _ in range(g_groups)]
            pending = [None]  # (g, f, tts) whose scores matmul is pending

            def emit_p1_tile(g, f):
                xt = p1.tile([128, 4, 512], BF16, tag="xt")
                nc.sync.dma_start(out=xt, in_=x_fm[g, :, f])
                tts = []
                for j in range(2):
                    h1 = psum.tile([128, 512], F32, tag="h1")
                    for k in range(4):
                        nc.tensor.matmul(
                            h1,
                            lhsT=w1_sb[:, k, j * 128 : (j + 1) * 128],
                            rhs=xt[:, k, :],
                            start=(k == 0),
                            stop=(k == 3),
                        )
                    tt = p1.tile([128, 512], BF16, tag=f"tt{j}")
                    nc.scalar.activation(tt, h1, Tanh, bias=b1_sb[:, j : j + 1])
                    tts.append(tt)
                if pending[0] is not None:
                    emit_scores(*pending[0])
                pending[0] = (g, f, tts)

            def flush_scores():
                if pending[0] is not None:
                    emit_scores(*pending[0])
                    pending[0] = None

            def seg_reduce_pre(s, vec_nm):
                """DVE part of sum_n st[n,s]*vec[n]: fills s['part']."""
                nc.vector.tensor_mul(
                    s["prod"], s["st"], vec_nm.to_broadcast([128, C, spg])
                )
                nc.vector.reduce_sum(
                    out=s["part"],
                    in_=s["prod"].rearrange("p c s -> p s c"),
                    axis=mybir.AxisListType.X,
                )

            def seg_reduce_mm(s):
                """PE part: cross-partition sum -> psum [spg, 1]."""
                ps = psum_small.tile([spg, 1], F32, tag="segred")
                nc.tensor.matmul(
                    ps, lhsT=s["part"], rhs=ones_sb, start=True, stop=True
                )
                return ps

            def replicate(g, col, idx, tag):
                """[spg, 1] col -> [128, spg] replicated across partitions:
                DVE 32x32 transpose to a row, then PE outer product with a
                ones column (no DRAM bounce -> ~3us less chain latency)."""
                c32 = mid.tile([32, 32], F32, tag="c32", name="c32_t")
                nc.vector.tensor_copy(c32[0:spg, 0:1], col)
                r32 = mid.tile([32, 32], F32, tag="r32", name="r32_t")
                nc.vector.transpose(r32, c32)
                rep_ps = psum_small.tile([128, spg], F32, tag="segred")
                nc.tensor.matmul(
                    rep_ps, lhsT=ones_row, rhs=r32[0:1, 0:spg],
                    start=True, stop=True,
                )
                rep = mid.tile([128, spg], F32, tag=tag)
                nc.vector.tensor_copy(rep, rep_ps)
                return rep

            def gather(s, rep, out_tag):
                """out[n] = sum_s st[n,s] * rep[:, s]  (one-hot gather)"""
                nc.vector.tensor_mul(
                    s["prod"],
                    s["st"],
                    rep.to_broadcast([128, spg, C]).rearrange("p s c -> p c s"),
                )
                o = mid.tile([128, C], F32, tag=out_tag)
                nc.vector.reduce_sum(out=o, in_=s["prod"], axis=mybir.AxisListType.X)
                return o

            def mid_A_pre(g):
                """Before the last score write: start the node-major scores
                read for everything already written (c < 4*(F-1)) so the DVE
                chain never blocks long on the final slice."""
                s = S[g]
                s["sc"] = mid.tile([128, C], F32, tag="sc", name="sc_t")
                cpre = 4 * (F - 1)
                nc.gpsimd.dma_start(
                    out=s["sc"][:, 0:cpre],
                    in_=sc_scratch[g, 0 : cpre * 128].rearrange("(c p) -> p c", p=128),
                )

            def mid_st_load(g):
                s = S[g]
                s["st"] = stp.tile([128, C, spg], F32, tag="st", name="st_t")
                nc.sync.dma_start(out=s["st"], in_=st_d[g])

            def mid_A(g):
                """After pass1(g) scores flushed: finish the scores read and
                prep the s_seg partial (all DVE/DMA)."""
                s = S[g]
                cpre = 4 * (F - 1)
                nc.gpsimd.dma_start(
                    out=s["sc"][:, cpre:C],
                    in_=sc_scratch[g, cpre * 128 :].rearrange("(c p) -> p c", p=128),
                )
                s["prod"] = mid.tile([128, C, spg], F32, tag="prod", name="prod_t")
                s["part"] = mid.tile([128, spg], F32, tag="part", name="part_t")
                seg_reduce_pre(s, s["sc"])

            def mid_B(g):
                """sseg matmul (PE) + everything up to the wseg partial."""
                s = S[g]
                sseg_ps = seg_reduce_mm(s)
                scol = mid.tile([spg, 1], F32, tag="scol")
                nc.vector.tensor_copy(scol, sseg_ps)
                srep = replicate(g, scol, 0, "srep")
                mx_nm = gather(s, srep, "mx")
                wd_nm = mid.tile([128, C], F32, tag="wd")
                nc.vector.tensor_sub(wd_nm, s["sc"], mx_nm)
                s["w"] = mid.tile([128, C], F32, tag="w", name="w_t")
                nc.scalar.activation(s["w"], wd_nm, Exp)
                seg_reduce_pre(s, s["w"])
                if dbg_d is not None:
                    nc.sync.dma_start(out=dbg_d[g, 0], in_=s["sc"])
                    nc.sync.dma_start(out=dbg_d[g, 1], in_=mx_nm)

            def mid_C(g):
                """wseg matmul (PE) + att + S_w^T."""
                s = S[g]
                wseg_ps = seg_reduce_mm(s)
                wcol = mid.tile([spg, 1], F32, tag="wcol")
                nc.vector.tensor_scalar_add(wcol, wseg_ps, 1e-8)
                rcol = mid.tile([spg, 1], F32, tag="rcol")
                nc.vector.reciprocal(rcol, wcol)
                rrep = replicate(g, rcol, 1, "rrep")
                den_nm = gather(s, rrep, "den")
                att_nm = mid.tile([128, C], F32, tag="att")
                nc.vector.tensor_mul(att_nm, s["w"], den_nm)
                s["swt"] = mid.tile([128, C, spg], BF16, tag="swt", name="swt_t")
                nc.vector.tensor_mul(
                    s["swt"], s["st"], att_nm.to_broadcast([128, C, spg])
                )
                if dbg_d is not None:
                    nc.sync.dma_start(out=dbg_d[g, 2], in_=s["w"])
                    nc.sync.dma_start(out=dbg_d[g, 3], in_=att_nm)

            def pass2_start(g):
                s = S[g]
                swt = s["swt"]
                dum_p2 = psum_small.tile([spg, 1], F32, tag="segred")
                nc.tensor.matmul(
                    dum_p2, lhsT=swt[:, 0, :], rhs=swt[:, 0, 0:1],
                    start=True, stop=True,
                )
                s["outp"] = psum_out.tile([spg, 512], F32, tag="outp", name="outp_t")

            def pass2_load(g, cb, tag="xt2"):
                s = S[g]
                xt2 = p2.tile([128, C_BATCH, HID], BF16, tag=tag, name="xt2_t")
                nc.sync.dma_start(
                    out=xt2, in_=x_nm[g][:, cb * C_BATCH : (cb + 1) * C_BATCH, :]
                )
                s.setdefault("xt2", {})[cb] = xt2

            def pass2_batch(g, cb):
                s = S[g]
                swt = s["swt"]
                xt2 = s["xt2"].pop(cb)
                for i in range(C_BATCH):
                    c = cb * C_BATCH + i
                    nc.tensor.matmul(
                        s["outp"],
                        lhsT=swt[:, c, :],
                        rhs=xt2[:, i, :],
                        start=(c == 0),
                        stop=(c == C - 1),
                    )

            def pass2_finish(g):
                s = S[g]
                out_sb = outp2.tile([spg, HID], F32, tag="out_sb")
                nc.vector.tensor_copy(out_sb, s["outp"])
                nc.gpsimd.dma_start(out=out_d[g * spg : (g + 1) * spg, :], in_=out_sb)
                s.clear()

            # Schedule: group g's middle PE matmuls hook late into pass1(g+1)
            # (the serial score->segment chain takes ~15us), and pass2(g)
            # interleaves with pass1(g+2) — so the LAST group's middle hides
            # under pass2(G-2) instead of stalling the tail.
            NB = C // C_BATCH  # pass-2 batches per group
            LA = 4  # pass-2 load lookahead (batches)
            p2_start_f = 4   # pass2(g-2) runs early: it has no fresh deps
            hook_b = max(p2_start_f + 3, (F * 7) // 13)   # middle chain ~11us
            hook_c = min(F - 2, hook_b + 4)
            tail_split = NB // 2

            def pass2_stretch(g, lo, hi):
                for cb in range(lo, hi):
                    if cb + LA < NB:
                        pass2_load(g, cb + LA)
                    pass2_batch(g, cb)

            for g0 in range(g_groups):
                mid_st_load(g0)
            for g in range(g_groups):
                last = g == g_groups - 1
                for f in range(F):
                    emit_p1_tile(g, f)
                    if g >= 2:
                        if f < p2_start_f:
                            pass2_load(g - 2, f)
                        elif f == p2_start_f:
                            pass2_start(g - 2)
                        else:
                            nb_here = tail_split if last else NB
                            lo = (f - p2_start_f - 1) * nb_here // (F - p2_start_f - 1)
                            hi = (f - p2_start_f) * nb_here // (F - p2_start_f - 1)
                            pass2_stretch(g - 2, lo, hi)
                    if g >= 1:
                        if f == hook_b:
                            mid_B(g - 1)
                        elif f == hook_c:
                            mid_C(g - 1)
                mid_A_pre(g)
                flush_scores()
                mid_A(g)
                if g >= 2 and g - 2 != g_groups - 3:
                    pass2_finish(g - 2)
            # tail
            gl = g_groups - 1
            pass2_stretch(gl - 2, tail_split, (tail_split + NB) // 2)
            mid_B(gl)
            pass2_stretch(gl - 2, (tail_split + NB) // 2, NB)
            mid_C(gl)
            pass2_finish(gl - 2)
            for cb in range(LA):
                pass2_load(gl - 1, cb)
            pass2_start(gl - 1)
            pass2_stretch(gl - 1, 0, NB)
            pass2_finish(gl - 1)
            for cb in range(LA):
                pass2_load(gl, cb)
            pass2_start(gl)
            pass2_stretch(gl, 0, NB)
            pass2_finish(gl)

    nc.compile()
    return nc


def _prepare(x, batch, W1, b1, W2, b2, g_groups):
    """Host-side sharding/packing.  Returns (in_maps, gmax, spg, b2val)."""
    x = np.ascontiguousarray(np.asarray(x, dtype=np.float32))
    batch = np.asarray(batch).astype(np.int64)
    spg = SEG_PER_CORE // g_groups

    bounds = np.searchsorted(batch, np.arange(NSEG + 1))
    glens = bounds[spg:NSEG + 1:spg] - bounds[0:NSEG:spg]  # len per (core,group)
    gmax = max(512, _round_up(int(glens.max()), 512))
    C = gmax // 128
    F = gmax // 512

    xb = x.astype(NPBF16)
    w1b = np.ascontiguousarray(np.asarray(W1, np.float32).astype(NPBF16))
    w2b = np.ascontiguousarray(np.asarray(W2, np.float32).astype(NPBF16).reshape(H2, 1))
    b1f = np.ascontiguousarray(np.asarray(b1, np.float32).reshape(H2))
    b2val = float(np.asarray(b2, np.float32).reshape(-1)[0])

    in_maps = []
    for core in range(NCORES):
        x_nm = np.zeros((g_groups, 128, C, HID), NPBF16)
        x_fm = np.zeros((g_groups, 128, F, 4, 512), NPBF16)
        st = np.zeros((g_groups, 128, C, spg), np.float32)
        for g in range(g_groups):
            s0 = core * SEG_PER_CORE + g * spg
            n0, n1 = int(bounds[s0]), int(bounds[s0 + spg])
            L = n1 - n0
            xg = np.zeros((gmax, HID), NPBF16)
            xg[:L] = xb[n0:n1]
            # node-major: [p, c, hid], node = c*128 + p
            x_nm[g] = xg.reshape(C, 128, HID).transpose(1, 0, 2)
            # feature-major: [p, f, k, n], hid = k*128 + p, node = f*512 + n
            xT = np.ascontiguousarray(xg.T)  # [HID, gmax]
            x_fm[g] = xT.reshape(4, 128, F, 512).transpose(1, 2, 0, 3)
            oh = np.zeros((gmax, spg), np.float32)
            oh[np.arange(L), (batch[n0:n1] - s0).astype(np.int64)] = 1.0
            st[g] = oh.reshape(C, 128, spg).transpose(1, 0, 2)
        in_maps.append(
            {
                "x_nm": x_nm,
                "x_fm": x_fm,
                "st": st,
                "w1": w1b,
                "b1": b1f,
                "w2": w2b,
            }
        )
    return in_maps, gmax, spg, b2val


def _run(inputs, trace=False, **run_kwargs):
    in_maps, gmax, spg, b2val = _prepare(
        inputs["x"], inputs["batch"], inputs["W1"], inputs["b1"],
        inputs["W2"], inputs["b2"], G,
    )
    nc = _build_graph(G, gmax, spg, b2val)
    res = run_bass_kernel_spmd(
        nc, in_maps, core_ids=list(range(NCORES)), trace=trace, **run_kwargs
    )
    out = np.concatenate([r["out"] for r in res.results], axis=0)
    return out.astype(np.float32), res


def kernel(**inputs) -> np.ndarray:
    out, _ = _run(inputs, trace=False)
    return out


# revision 77
# speedup vs baseline: 1.0338x; 1.0338x over previous
"""AttentionPooling (segment softmax-pool) Trainium2 kernel, 8 NeuronCores.

Sharding: each core owns 32 consecutive segments (contiguous node range since
`batch` is sorted); the 32 segments split into G groups of spg segments whose
node ranges are padded to a fixed Gmax so all cores/groups run one static
graph.  All segment reductions are core-local; no collectives.

Per-core, per-group pipeline (matmuls bf16 in / fp32 accum):
  pass 1 (feature-major): scoresT = tanh(W1^T @ x^T + b1)^T @ W2 + b2
  middle: one-hot segment sums (DVE mult+reduce + PE cross-partition sum),
          softmax weights per the reference's scatter-add "max" stabilizer
          and the 1e-8 epsilon
  pass 2 (node-major):   out[seg, :] = (S^T * att)^T @ x   via PE accumulation

x is uploaded twice (node-major and feature-major, both bf16, partition-major
packed so every DMA reads >=4KB contiguous per partition); no on-chip
transposes.  Groups pipeline: group g's serial middle hides under group g+1's
pass-1 matmuls.
"""

import functools
import sys

sys.path.insert(0, "/opt/trn_rl_repo")

import ml_dtypes
import numpy as np

import concourse.bass as bass
import concourse.tile as tile
from concourse import bacc, mybir
from concourse.bass_utils import run_bass_kernel_spmd

NCORES = 8
NSEG = 256
HID = 512
H2 = 256  # hidden//2
SEG_PER_CORE = NSEG // NCORES  # 32

BF16 = mybir.dt.bfloat16
F32 = mybir.dt.float32
NPBF16 = ml_dtypes.bfloat16

G = 4  # groups per core
C_BATCH = 4  # pass-2 node chunks per DMA

DEBUG_TAPS = False


def _round_up(v, m):
    return (v + m - 1) // m * m


@functools.lru_cache(maxsize=4)
def _build_graph(g_groups, gmax, spg, b2val):
    C = gmax // 128  # node chunks per group
    F = gmax // 512  # pass-1 free-dim tiles per group

    nc = bacc.Bacc(None, target_bir_lowering=False, debug=False)
    # partition-major packed: every DMA reads contiguous bytes per partition
    x_nm = nc.declare_dram_parameter("x_nm", [g_groups, 128, C, HID], BF16, isOutput=False)
    x_fm = nc.declare_dram_parameter("x_fm", [g_groups, 128, F, 4, 512], BF16, isOutput=False)
    st_d = nc.declare_dram_parameter("st", [g_groups, 128, C, spg], F32, isOutput=False)
    w1_d = nc.declare_dram_parameter("w1", [HID, H2], BF16, isOutput=False)
    b1_d = nc.declare_dram_parameter("b1", [H2], F32, isOutput=False)
    w2_d = nc.declare_dram_parameter("w2", [H2, 1], BF16, isOutput=False)
    out_d = nc.declare_dram_parameter("out", [g_groups * spg, HID], F32, isOutput=True)
    seg_scratch = nc.dram_tensor("seg_scratch", [g_groups, 2, spg], F32)
    sc_scratch = nc.dram_tensor("sc_scratch", [g_groups, gmax], F32)
    dbg_d = None
    if DEBUG_TAPS:
        dbg_d = nc.declare_dram_parameter(
            "dbg", [g_groups, 4, 128, C], F32, isOutput=True
        )

    Tanh = mybir.ActivationFunctionType.Tanh
    Exp = mybir.ActivationFunctionType.Exp
    Copy = mybir.ActivationFunctionType.Copy

    with tile.TileContext(nc) as tc:
        with (
            tc.tile_pool(name="consts", bufs=1) as consts,
            tc.tile_pool(name="p1", bufs=13) as p1,
            tc.tile_pool(name="mid", bufs=4) as mid,
            tc.tile_pool(name="stp", bufs=4) as stp,
            tc.tile_pool(name="p2", bufs=8) as p2,
            tc.tile_pool(name="outp2", bufs=2) as outp2,
            tc.tile_pool(name="psum", bufs=3, space="PSUM") as psum,
            tc.tile_pool(name="psum_scp", bufs=2, space="PSUM") as psum_scp,
            tc.tile_pool(name="psum_small", bufs=1, space="PSUM") as psum_small,
            tc.tile_pool(name="psum_out", bufs=2, space="PSUM") as psum_out,
        ):
            # constants
            w1_sb = consts.tile([128, 4, H2], BF16)  # [p, kchunk, j]
            nc.sync.dma_start(out=w1_sb, in_=w1_d[:].rearrange("(k p) j -> p k j", p=128))
            b1_sb = consts.tile([128, 2], F32)  # [p, jchunk]
            nc.sync.dma_start(out=b1_sb, in_=b1_d[:].rearrange("(j p) -> p j", p=128))
            w2_sb = consts.tile([128, 2], BF16)
            nc.sync.dma_start(out=w2_sb, in_=w2_d[:].rearrange("(j p) o -> p (j o)", p=128))
            ones_sb = consts.tile([128, 1], F32)
            nc.vector.memset(ones_sb, 1.0)
            ones_row = consts.tile([1, 128], F32)
            nc.vector.memset(ones_row, 1.0)

            # engine pre-touches: keep later compute instructions at <=1
            # foreign wait and off the event-semaphore slow path.
            dum_act = consts.tile([128, 1], F32)
            nc.scalar.activation(dum_act, b1_sb[:, 0:1], Copy)
            dum_ps = psum_small.tile([128, 1], F32, tag="segred")
            nc.tensor.matmul(dum_ps, lhsT=w1_sb[:, 0, 0:128], rhs=w1_sb[:, 0, 0:1],
                             start=True, stop=True)
            dum_ps2 = psum_small.tile([1, 1], F32, tag="segred")
            nc.tensor.matmul(dum_ps2, lhsT=w2_sb[:, 0:1], rhs=w2_sb[:, 0:1],
                             start=True, stop=True)

            def emit_scores(g, f, tts):
                """second MLP layer + score write-out for tile f of group g;
                emitted one iteration late so PE never stalls on the tanh."""
                scp = psum_scp.tile([1, 512], F32, tag="scp")
                for j in range(2):
                    nc.tensor.matmul(
                        scp,
                        lhsT=w2_sb[:, j : j + 1],
                        rhs=tts[j],
                        start=(j == 0),
                        stop=(j == 1),
                    )
                srow = p1.tile([1, 512], F32, tag="srow")
                nc.vector.tensor_scalar_add(srow, scp, b2val)
                nc.gpsimd.dma_start(
                    out=sc_scratch[g, f * 512 : (f + 1) * 512][None, :],
                    in_=srow[:, :],
                )

            # Per-group state for the software-pipelined schedule.
            S = [dict() for _ in range(g_groups)]
            pending = [None]  # (g, f, tts) whose scores matmul is pending

            def emit_p1_tile(g, f):
                xt = p1.tile([128, 4, 512], BF16, tag="xt")
                nc.sync.dma_start(out=xt, in_=x_fm[g, :, f])
                tts = []
                for j in range(2):
                    h1 = psum.tile([128, 512], F32, tag="h1")
                    for k in range(4):
                        nc.tensor.matmul(
                            h1,
                            lhsT=w1_sb[:, k, j * 128 : (j + 1) * 128],
                            rhs=xt[:, k, :],
                            start=(k == 0),
                            stop=(k == 3),
                        )
                    tt = p1.tile([128, 512], BF16, tag=f"tt{j}")
                    nc.scalar.activation(tt, h1, Tanh, bias=b1_sb[:, j : j + 1])
                    tts.append(tt)
                if pending[0] is not None:
                    emit_scores(*pending[0])
                pending[0] = (g, f, tts)

            def flush_scores():
                if pending[0] is not None:
                    emit_scores(*pending[0])
                    pending[0] = None

            def seg_reduce_pre(s, vec_nm):
                """DVE part of sum_n st[n,s]*vec[n]: fills s['part']."""
                nc.vector.tensor_mul(
                    s["prod"], s["st"], vec_nm.to_broadcast([128, C, spg])
                )
                nc.vector.reduce_sum(
                    out=s["part"],
                    in_=s["prod"].rearrange("p c s -> p s c"),
                    axis=mybir.AxisListType.X,
                )

            def seg_reduce_mm(s):
                """PE part: cross-partition sum -> psum [spg, 1]."""
                ps = psum_small.tile([spg, 1], F32, tag="segred")
                nc.tensor.matmul(
                    ps, lhsT=s["part"], rhs=ones_sb, start=True, stop=True
                )
                return ps

            def replicate(g, col, idx, tag):
                """[spg, 1] col -> [128, spg] replicated across partitions:
                DVE 32x32 transpose to a row, then PE outer product with a
                ones column (no DRAM bounce -> ~3us less chain latency)."""
                c32 = mid.tile([32, 32], F32, tag="c32", name="c32_t")
                nc.vector.tensor_copy(c32[0:spg, 0:1], col)
                r32 = mid.tile([32, 32], F32, tag="r32", name="r32_t")
                nc.vector.transpose(r32, c32)
                rep_ps = psum_small.tile([128, spg], F32, tag="segred")
                nc.tensor.matmul(
                    rep_ps, lhsT=ones_row, rhs=r32[0:1, 0:spg],
                    start=True, stop=True,
                )
                rep = mid.tile([128, spg], F32, tag=tag)
                nc.vector.tensor_copy(rep, rep_ps)
                return rep

            def gather(s, rep, out_tag):
                """out[n] = sum_s st[n,s] * rep[:, s]  (one-hot gather)"""
                nc.vector.tensor_mul(
                    s["prod"],
                    s["st"],
                    rep.to_broadcast([128, spg, C]).rearrange("p s c -> p c s"),
                )
                o = mid.tile([128, C], F32, tag=out_tag)
                nc.vector.reduce_sum(out=o, in_=s["prod"], axis=mybir.AxisListType.X)
                return o

            def mid_A_pre(g):
                """Before the last score write: start the node-major scores
                read for everything already written (c < 4*(F-1)) so the DVE
                chain never blocks long on the final slice."""
                s = S[g]
                s["sc"] = mid.tile([128, C], F32, tag="sc", name="sc_t")
                cpre = 4 * (F - 1)
                nc.gpsimd.dma_start(
                    out=s["sc"][:, 0:cpre],
                    in_=sc_scratch[g, 0 : cpre * 128].rearrange("(c p) -> p c", p=128),
                )

            def mid_st_load(g):
                s = S[g]
                s["st"] = stp.tile([128, C, spg], F32, tag="st", name="st_t")
                nc.gpsimd.dma_start(out=s["st"], in_=st_d[g])

            def mid_A(g):
                """After pass1(g) scores flushed: finish the scores read and
                prep the s_seg partial (all DVE/DMA)."""
                s = S[g]
                cpre = 4 * (F - 1)
                nc.gpsimd.dma_start(
                    out=s["sc"][:, cpre:C],
                    in_=sc_scratch[g, cpre * 128 :].rearrange("(c p) -> p c", p=128),
                )
                s["prod"] = mid.tile([128, C, spg], F32, tag="prod", name="prod_t")
                s["part"] = mid.tile([128, spg], F32, tag="part", name="part_t")
                seg_reduce_pre(s, s["sc"])

            def mid_B(g):
                """sseg matmul (PE) + everything up to the wseg partial."""
                s = S[g]
                sseg_ps = seg_reduce_mm(s)
                scol = mid.tile([spg, 1], F32, tag="scol")
                nc.vector.tensor_copy(scol, sseg_ps)
                srep = replicate(g, scol, 0, "srep")
                mx_nm = gather(s, srep, "mx")
                wd_nm = mid.tile([128, C], F32, tag="wd")
                nc.vector.tensor_sub(wd_nm, s["sc"], mx_nm)
                s["w"] = mid.tile([128, C], F32, tag="w", name="w_t")
                nc.scalar.activation(s["w"], wd_nm, Exp)
                seg_reduce_pre(s, s["w"])
                if dbg_d is not None:
                    nc.sync.dma_start(out=dbg_d[g, 0], in_=s["sc"])
                    nc.sync.dma_start(out=dbg_d[g, 1], in_=mx_nm)

            def mid_C(g):
                """wseg matmul (PE) + att + S_w^T."""
                s = S[g]
                wseg_ps = seg_reduce_mm(s)
                wcol = mid.tile([spg, 1], F32, tag="wcol")
                nc.vector.tensor_scalar_add(wcol, wseg_ps, 1e-8)
                rcol = mid.tile([spg, 1], F32, tag="rcol")
                nc.vector.reciprocal(rcol, wcol)
                rrep = replicate(g, rcol, 1, "rrep")
                den_nm = gather(s, rrep, "den")
                att_nm = mid.tile([128, C], F32, tag="att")
                nc.vector.tensor_mul(att_nm, s["w"], den_nm)
                s["swt"] = mid.tile([128, C, spg], BF16, tag="swt", name="swt_t")
                nc.vector.tensor_mul(
                    s["swt"], s["st"], att_nm.to_broadcast([128, C, spg])
                )
                if dbg_d is not None:
                    nc.sync.dma_start(out=dbg_d[g, 2], in_=s["w"])
                    nc.sync.dma_start(out=dbg_d[g, 3], in_=att_nm)

            def pass2_start(g):
                s = S[g]
                swt = s["swt"]
                dum_p2 = psum_small.tile([spg, 1], F32, tag="segred")
                nc.tensor.matmul(
                    dum_p2, lhsT=swt[:, 0, :], rhs=swt[:, 0, 0:1],
                    start=True, stop=True,
                )
                s["outp"] = psum_out.tile([spg, 512], F32, tag="outp", name="outp_t")

            def pass2_load(g, cb, tag="xt2"):
                s = S[g]
                xt2 = p2.tile([128, C_BATCH, HID], BF16, tag=tag, name="xt2_t")
                nc.sync.dma_start(
                    out=xt2, in_=x_nm[g][:, cb * C_BATCH : (cb + 1) * C_BATCH, :]
                )
                s.setdefault("xt2", {})[cb] = xt2

            def pass2_batch(g, cb):
                s = S[g]
                swt = s["swt"]
                xt2 = s["xt2"].pop(cb)
                for i in range(C_BATCH):
                    c = cb * C_BATCH + i
                    nc.tensor.matmul(
                        s["outp"],
                        lhsT=swt[:, c, :],
                        rhs=xt2[:, i, :],
                        start=(c == 0),
                        stop=(c == C - 1),
                    )

            def pass2_finish(g):
                s = S[g]
                out_sb = outp2.tile([spg, HID], F32, tag="out_sb")
                nc.vector.tensor_copy(out_sb, s["outp"])
                nc.gpsimd.dma_start(out=out_d[g * spg : (g + 1) * spg, :], in_=out_sb)
                s.clear()

            # Schedule: group g's middle PE matmuls hook late into pass1(g+1)
            # (the serial score->segment chain takes ~15us), and pass2(g)
            # interleaves with pass1(g+2) — so the LAST group's middle hides
            # under pass2(G-2) instead of stalling the tail.
            NB = C // C_BATCH  # pass-2 batches per group
            LA = 4  # pass-2 load lookahead (batches)
            p2_start_f = 4   # pass2(g-2) runs early: it has no fresh deps
            hook_b = max(p2_start_f + 3, (F * 7) // 13)   # middle chain ~11us
            hook_c = min(F - 2, hook_b + 4)
            tail_split = NB // 2

            def pass2_stretch(g, lo, hi):
                for cb in range(lo, hi):
                    if cb + LA < NB:
                        pass2_load(g, cb + LA)
                    pass2_batch(g, cb)

            for g in range(g_groups):
                last = g == g_groups - 1
                for f in range(F):
                    emit_p1_tile(g, f)
                    if f == 2:
                        mid_st_load(g)
                    if g >= 2:
                        if f < p2_start_f:
                            pass2_load(g - 2, f)
                        elif f == p2_start_f:
                            pass2_start(g - 2)
                        else:
                            nb_here = tail_split if last else NB
                            lo = (f - p2_start_f - 1) * nb_here // (F - p2_start_f - 1)
                            hi = (f - p2_start_f) * nb_here // (F - p2_start_f - 1)
                            pass2_stretch(g - 2, lo, hi)
                    if g >= 1:
                        if f == hook_b:
                            mid_B(g - 1)
                        elif f == hook_c:
                            mid_C(g - 1)
                mid_A_pre(g)
                flush_scores()
                mid_A(g)
                if g >= 2 and g - 2 != g_groups - 3:
                    pass2_finish(g - 2)
            # tail
            gl = g_groups - 1
            pass2_stretch(gl - 2, tail_split, (tail_split + NB) // 2)
            mid_B(gl)
            pass2_stretch(gl - 2, (tail_split + NB) // 2, NB)
            mid_C(gl)
            pass2_finish(gl - 2)
            for cb in range(LA):
                pass2_load(gl - 1, cb)
            pass2_start(gl - 1)
            pass2_stretch(gl - 1, 0, NB)
            pass2_finish(gl - 1)
            for cb in range(LA):
                pass2_load(gl, cb)
            pass2_start(gl)
            pass2_stretch(gl, 0, NB)
            pass2_finish(gl)

    nc.compile()
    return nc


def _prepare(x, batch, W1, b1, W2, b2, g_groups):
    """Host-side sharding/packing.  Returns (in_maps, gmax, spg, b2val)."""
    x = np.ascontiguousarray(np.asarray(x, dtype=np.float32))
    batch = np.asarray(batch).astype(np.int64)
    spg = SEG_PER_CORE // g_groups

    bounds = np.searchsorted(batch, np.arange(NSEG + 1))
    glens = bounds[spg:NSEG + 1:spg] - bounds[0:NSEG:spg]  # len per (core,group)
    gmax = max(512, _round_up(int(glens.max()), 512))
    C = gmax // 128
    F = gmax // 512

    xb = x.astype(NPBF16)
    w1b = np.ascontiguousarray(np.asarray(W1, np.float32).astype(NPBF16))
    w2b = np.ascontiguousarray(np.asarray(W2, np.float32).astype(NPBF16).reshape(H2, 1))
    b1f = np.ascontiguousarray(np.asarray(b1, np.float32).reshape(H2))
    b2val = float(np.asarray(b2, np.float32).reshape(-1)[0])

    in_maps = []
    for core in range(NCORES):
        x_nm = np.zeros((g_groups, 128, C, HID), NPBF16)
        x_fm = np.zeros((g_groups, 128, F, 4, 512), NPBF16)
        st = np.zeros((g_groups, 128, C, spg), np.float32)
        for g in range(g_groups):
            s0 = core * SEG_PER_CORE + g * spg
            n0, n1 = int(bounds[s0]), int(bounds[s0 + spg])
            L = n1 - n0
            xg = np.zeros((gmax, HID), NPBF16)
            xg[:L] = xb[n0:n1]
            # node-major: [p, c, hid], node = c*128 + p
            x_nm[g] = xg.reshape(C, 128, HID).transpose(1, 0, 2)
            # feature-major: [p, f, k, n], hid = k*128 + p, node = f*512 + n
            xT = np.ascontiguousarray(xg.T)  # [HID, gmax]
            x_fm[g] = xT.reshape(4, 128, F, 512).transpose(1, 2, 0, 3)
            oh = np.zeros((gmax, spg), np.float32)
            oh[np.arange(L), (batch[n0:n1] - s0).astype(np.int64)] = 1.0
            st[g] = oh.reshape(C, 128, spg).transpose(1, 0, 2)
        in_maps.append(
            {
                "x_nm": x_nm,
                "x_fm": x_fm,
                "st": st,
                "w1": w1b,
                "b1": b1f,
                "w2": w2b,
            }
        )
    return in_maps, gmax, spg, b2val


def _run(inputs, trace=False, **run_kwargs):
    in_maps, gmax, spg, b2val = _prepare(
        inputs["x"], inputs["batch"], inputs["W1"], inputs["b1"],
        inputs["W2"], inputs["b2"], G,
    )
    nc = _build_graph(G, gmax, spg, b2val)
    res = run_bass_kernel_spmd(
        nc, in_maps, core_ids=list(range(NCORES)), trace=trace, **run_kwargs
    )
    out = np.concatenate([r["out"] for r in res.results], axis=0)
    return out.astype(np.float32), res


def kernel(**inputs) -> np.ndarray:
    out, _ = _run(inputs, trace=False)
    return out


# revision 78
# speedup vs baseline: 1.0801x; 1.0448x over previous
"""AttentionPooling (segment softmax-pool) Trainium2 kernel, 8 NeuronCores.

Sharding: each core owns 32 consecutive segments (contiguous node range since
`batch` is sorted); the 32 segments split into G groups of spg segments whose
node ranges are padded to a fixed Gmax so all cores/groups run one static
graph.  All segment reductions are core-local; no collectives.

Per-core, per-group pipeline (matmuls bf16 in / fp32 accum):
  pass 1 (feature-major): scoresT = tanh(W1^T @ x^T + b1)^T @ W2 + b2
  middle: one-hot segment sums (DVE mult+reduce + PE cross-partition sum),
          softmax weights per the reference's scatter-add "max" stabilizer
          and the 1e-8 epsilon
  pass 2 (node-major):   out[seg, :] = (S^T * att)^T @ x   via PE accumulation

x is uploaded twice (node-major and feature-major, both bf16, partition-major
packed so every DMA reads >=4KB contiguous per partition); no on-chip
transposes.  Groups pipeline: group g's serial middle hides under group g+1's
pass-1 matmuls.
"""

import functools
import sys

sys.path.insert(0, "/opt/trn_rl_repo")

import ml_dtypes
import numpy as np

import concourse.bass as bass
import concourse.tile as tile
from concourse import bacc, mybir
from concourse.bass_utils import run_bass_kernel_spmd

NCORES = 8
NSEG = 256
HID = 512
H2 = 256  # hidden//2
SEG_PER_CORE = NSEG // NCORES  # 32

BF16 = mybir.dt.bfloat16
F32 = mybir.dt.float32
NPBF16 = ml_dtypes.bfloat16

G = 4  # groups per core
C_BATCH = 4  # pass-2 node chunks per DMA

DEBUG_TAPS = False


def _round_up(v, m):
    return (v + m - 1) // m * m


@functools.lru_cache(maxsize=4)
def _build_graph(g_groups, gmax, spg, b2val):
    C = gmax // 128  # node chunks per group
    F = gmax // 512  # pass-1 free-dim tiles per group

    nc = bacc.Bacc(None, target_bir_lowering=False, debug=False)
    # partition-major packed: every DMA reads contiguous bytes per partition
    x_nm = nc.declare_dram_parameter("x_nm", [g_groups, 128, C, HID], BF16, isOutput=False)
    x_fm = nc.declare_dram_parameter("x_fm", [g_groups, 128, F, 4, 512], BF16, isOutput=False)
    st_d = nc.declare_dram_parameter("st", [g_groups, 128, C, spg], F32, isOutput=False)
    w1_d = nc.declare_dram_parameter("w1", [HID, H2], BF16, isOutput=False)
    b1_d = nc.declare_dram_parameter("b1", [H2], F32, isOutput=False)
    w2_d = nc.declare_dram_parameter("w2", [H2, 1], BF16, isOutput=False)
    out_d = nc.declare_dram_parameter("out", [g_groups * spg, HID], F32, isOutput=True)
    seg_scratch = nc.dram_tensor("seg_scratch", [g_groups, 2, spg], F32)
    sc_scratch = nc.dram_tensor("sc_scratch", [g_groups, gmax], F32)
    dbg_d = None
    if DEBUG_TAPS:
        dbg_d = nc.declare_dram_parameter(
            "dbg", [g_groups, 4, 128, C], F32, isOutput=True
        )

    Tanh = mybir.ActivationFunctionType.Tanh
    Exp = mybir.ActivationFunctionType.Exp
    Copy = mybir.ActivationFunctionType.Copy

    with tile.TileContext(nc) as tc:
        with (
            tc.tile_pool(name="consts", bufs=1) as consts,
            tc.tile_pool(name="p1", bufs=13) as p1,
            tc.tile_pool(name="mid", bufs=4) as mid,
            tc.tile_pool(name="stp", bufs=4) as stp,
            tc.tile_pool(name="p2", bufs=8) as p2,
            tc.tile_pool(name="outp2", bufs=2) as outp2,
            tc.tile_pool(name="psum", bufs=3, space="PSUM") as psum,
            tc.tile_pool(name="psum_scp", bufs=2, space="PSUM") as psum_scp,
            tc.tile_pool(name="psum_small", bufs=1, space="PSUM") as psum_small,
            tc.tile_pool(name="psum_out", bufs=2, space="PSUM") as psum_out,
        ):
            # constants
            w1_sb = consts.tile([128, 4, H2], BF16)  # [p, kchunk, j]
            nc.sync.dma_start(out=w1_sb, in_=w1_d[:].rearrange("(k p) j -> p k j", p=128))
            b1_sb = consts.tile([128, 2], F32)  # [p, jchunk]
            nc.sync.dma_start(out=b1_sb, in_=b1_d[:].rearrange("(j p) -> p j", p=128))
            w2_sb = consts.tile([128, 2], BF16)
            nc.sync.dma_start(out=w2_sb, in_=w2_d[:].rearrange("(j p) o -> p (j o)", p=128))
            ones_sb = consts.tile([128, 1], F32)
            nc.vector.memset(ones_sb, 1.0)
            ones_row = consts.tile([1, 128], F32)
            nc.vector.memset(ones_row, 1.0)

            # engine pre-touches: keep later compute instructions at <=1
            # foreign wait and off the event-semaphore slow path.
            dum_act = consts.tile([128, 1], F32)
            nc.scalar.activation(dum_act, b1_sb[:, 0:1], Copy)
            dum_ps = psum_small.tile([128, 1], F32, tag="segred")
            nc.tensor.matmul(dum_ps, lhsT=w1_sb[:, 0, 0:128], rhs=w1_sb[:, 0, 0:1],
                             start=True, stop=True)
            dum_ps2 = psum_small.tile([1, 1], F32, tag="segred")
            nc.tensor.matmul(dum_ps2, lhsT=w2_sb[:, 0:1], rhs=w2_sb[:, 0:1],
                             start=True, stop=True)

            def emit_scores(g, f, tts):
                """second MLP layer + score write-out for tile f of group g;
                emitted one iteration late so PE never stalls on the tanh."""
                scp = psum_scp.tile([1, 512], F32, tag="scp")
                for j in range(2):
                    nc.tensor.matmul(
                        scp,
                        lhsT=w2_sb[:, j : j + 1],
                        rhs=tts[j],
                        start=(j == 0),
                        stop=(j == 1),
                    )
                srow = p1.tile([1, 512], F32, tag="srow")
                nc.vector.tensor_scalar_add(srow, scp, b2val)
                nc.gpsimd.dma_start(
                    out=sc_scratch[g, f * 512 : (f + 1) * 512][None, :],
                    in_=srow[:, :],
                )

            # Per-group state for the software-pipelined schedule.
            S = [dict() for _ in range(g_groups)]
            pending = [None]  # (g, f, tts) whose scores matmul is pending

            def emit_p1_tile(g, f):
                xt = p1.tile([128, 4, 512], BF16, tag="xt")
                nc.sync.dma_start(out=xt, in_=x_fm[g, :, f])
                tts = []
                for j in range(2):
                    h1 = psum.tile([128, 512], F32, tag="h1")
                    for k in range(4):
                        nc.tensor.matmul(
                            h1,
                            lhsT=w1_sb[:, k, j * 128 : (j + 1) * 128],
                            rhs=xt[:, k, :],
                            start=(k == 0),
                            stop=(k == 3),
                        )
                    tt = p1.tile([128, 512], BF16, tag=f"tt{j}")
                    nc.scalar.activation(tt, h1, Tanh, bias=b1_sb[:, j : j + 1])
                    tts.append(tt)
                if pending[0] is not None:
                    emit_scores(*pending[0])
                pending[0] = (g, f, tts)

            def flush_scores():
                if pending[0] is not None:
                    emit_scores(*pending[0])
                    pending[0] = None

            def seg_reduce_pre(s, vec_nm):
                """DVE part of sum_n st[n,s]*vec[n]: fills s['part']."""
                nc.vector.tensor_mul(
                    s["prod"], s["st"], vec_nm.to_broadcast([128, C, spg])
                )
                nc.vector.reduce_sum(
                    out=s["part"],
                    in_=s["prod"].rearrange("p c s -> p s c"),
                    axis=mybir.AxisListType.X,
                )

            def seg_reduce_mm(s):
                """PE part: cross-partition sum -> psum [spg, 1]."""
                ps = psum_small.tile([spg, 1], F32, tag="segred")
                nc.tensor.matmul(
                    ps, lhsT=s["part"], rhs=ones_sb, start=True, stop=True
                )
                return ps

            def replicate(g, col, idx, tag):
                """[spg, 1] col -> [128, spg] replicated across partitions:
                DVE 32x32 transpose to a row, then PE outer product with a
                ones column (no DRAM bounce -> ~3us less chain latency)."""
                c32 = mid.tile([32, 32], F32, tag="c32", name="c32_t")
                nc.vector.tensor_copy(c32[0:spg, 0:1], col)
                r32 = mid.tile([32, 32], F32, tag="r32", name="r32_t")
                nc.vector.transpose(r32, c32)
                rep_ps = psum_small.tile([128, spg], F32, tag="segred")
                nc.tensor.matmul(
                    rep_ps, lhsT=ones_row, rhs=r32[0:1, 0:spg],
                    start=True, stop=True,
                )
                rep = mid.tile([128, spg], F32, tag=tag)
                nc.vector.tensor_copy(rep, rep_ps)
                return rep

            def gather(s, rep, out_tag):
                """out[n] = sum_s st[n,s] * rep[:, s]  (one-hot gather)"""
                nc.vector.tensor_mul(
                    s["prod"],
                    s["st"],
                    rep.to_broadcast([128, spg, C]).rearrange("p s c -> p c s"),
                )
                o = mid.tile([128, C], F32, tag=out_tag)
                nc.vector.reduce_sum(out=o, in_=s["prod"], axis=mybir.AxisListType.X)
                return o

            def mid_A_pre(g):
                """Before the last score write: start the node-major scores
                read for everything already written (c < 4*(F-1)) so the DVE
                chain never blocks long on the final slice."""
                s = S[g]
                s["sc"] = mid.tile([128, C], F32, tag="sc", name="sc_t")
                cpre = 4 * (F - 1)
                nc.gpsimd.dma_start(
                    out=s["sc"][:, 0:cpre],
                    in_=sc_scratch[g, 0 : cpre * 128].rearrange("(c p) -> p c", p=128),
                )

            def mid_st_load(g):
                s = S[g]
                s["st"] = stp.tile([128, C, spg], F32, tag="st", name="st_t")
                nc.sync.dma_start(out=s["st"], in_=st_d[g])

            def mid_A(g):
                """After pass1(g) scores flushed: finish the scores read and
                prep the s_seg partial (all DVE/DMA)."""
                s = S[g]
                cpre = 4 * (F - 1)
                nc.gpsimd.dma_start(
                    out=s["sc"][:, cpre:C],
                    in_=sc_scratch[g, cpre * 128 :].rearrange("(c p) -> p c", p=128),
                )
                s["prod"] = mid.tile([128, C, spg], F32, tag="prod", name="prod_t")
                s["part"] = mid.tile([128, spg], F32, tag="part", name="part_t")
                seg_reduce_pre(s, s["sc"])

            def mid_B(g):
                """sseg matmul (PE) + everything up to the wseg partial."""
                s = S[g]
                sseg_ps = seg_reduce_mm(s)
                scol = mid.tile([spg, 1], F32, tag="scol")
                nc.vector.tensor_copy(scol, sseg_ps)
                srep = replicate(g, scol, 0, "srep")
                mx_nm = gather(s, srep, "mx")
                wd_nm = mid.tile([128, C], F32, tag="wd")
                nc.vector.tensor_sub(wd_nm, s["sc"], mx_nm)
                s["w"] = mid.tile([128, C], F32, tag="w", name="w_t")
                nc.scalar.activation(s["w"], wd_nm, Exp)
                seg_reduce_pre(s, s["w"])
                if dbg_d is not None:
                    nc.sync.dma_start(out=dbg_d[g, 0], in_=s["sc"])
                    nc.sync.dma_start(out=dbg_d[g, 1], in_=mx_nm)

            def mid_C(g):
                """wseg matmul (PE) + att + S_w^T."""
                s = S[g]
                wseg_ps = seg_reduce_mm(s)
                wcol = mid.tile([spg, 1], F32, tag="wcol")
                nc.vector.tensor_scalar_add(wcol, wseg_ps, 1e-8)
                rcol = mid.tile([spg, 1], F32, tag="rcol")
                nc.vector.reciprocal(rcol, wcol)
                rrep = replicate(g, rcol, 1, "rrep")
                den_nm = gather(s, rrep, "den")
                att_nm = mid.tile([128, C], F32, tag="att")
                nc.vector.tensor_mul(att_nm, s["w"], den_nm)
                s["swt"] = mid.tile([128, C, spg], BF16, tag="swt", name="swt_t")
                nc.vector.tensor_mul(
                    s["swt"], s["st"], att_nm.to_broadcast([128, C, spg])
                )
                if dbg_d is not None:
                    nc.sync.dma_start(out=dbg_d[g, 2], in_=s["w"])
                    nc.sync.dma_start(out=dbg_d[g, 3], in_=att_nm)

            def pass2_start(g):
                s = S[g]
                swt = s["swt"]
                dum_p2 = psum_small.tile([spg, 1], F32, tag="segred")
                nc.tensor.matmul(
                    dum_p2, lhsT=swt[:, 0, :], rhs=swt[:, 0, 0:1],
                    start=True, stop=True,
                )
                s["outp"] = psum_out.tile([spg, 512], F32, tag="outp", name="outp_t")

            def pass2_load(g, cb, tag="xt2"):
                s = S[g]
                xt2 = p2.tile([128, C_BATCH, HID], BF16, tag=tag, name="xt2_t")
                nc.sync.dma_start(
                    out=xt2, in_=x_nm[g][:, cb * C_BATCH : (cb + 1) * C_BATCH, :]
                )
                s.setdefault("xt2", {})[cb] = xt2

            def pass2_batch(g, cb):
                s = S[g]
                swt = s["swt"]
                xt2 = s["xt2"].pop(cb)
                for i in range(C_BATCH):
                    c = cb * C_BATCH + i
                    nc.tensor.matmul(
                        s["outp"],
                        lhsT=swt[:, c, :],
                        rhs=xt2[:, i, :],
                        start=(c == 0),
                        stop=(c == C - 1),
                    )

            def pass2_finish(g):
                s = S[g]
                out_sb = outp2.tile([spg, HID], F32, tag="out_sb")
                nc.vector.tensor_copy(out_sb, s["outp"])
                nc.gpsimd.dma_start(out=out_d[g * spg : (g + 1) * spg, :], in_=out_sb)
                s.clear()

            # Schedule: group g's middle PE matmuls hook late into pass1(g+1)
            # (the serial score->segment chain takes ~15us), and pass2(g)
            # interleaves with pass1(g+2) — so the LAST group's middle hides
            # under pass2(G-2) instead of stalling the tail.
            NB = C // C_BATCH  # pass-2 batches per group
            LA = 4  # pass-2 load lookahead (batches)
            p2_start_f = 4   # pass2(g-2) runs early: it has no fresh deps
            hook_b = max(p2_start_f + 3, (F * 7) // 13)   # middle chain ~11us
            hook_c = min(F - 2, hook_b + 4)
            tail_split = NB // 2

            def pass2_stretch(g, lo, hi):
                for cb in range(lo, hi):
                    if cb + LA < NB:
                        pass2_load(g, cb + LA)
                    pass2_batch(g, cb)

            for g in range(g_groups):
                last = g == g_groups - 1
                for f in range(F):
                    emit_p1_tile(g, f)
                    if f == 2:
                        mid_st_load(g)
                    if g >= 2:
                        if f < p2_start_f:
                            pass2_load(g - 2, f)
                        elif f == p2_start_f:
                            pass2_start(g - 2)
                        else:
                            nb_here = tail_split if last else NB
                            lo = (f - p2_start_f - 1) * nb_here // (F - p2_start_f - 1)
                            hi = (f - p2_start_f) * nb_here // (F - p2_start_f - 1)
                            pass2_stretch(g - 2, lo, hi)
                    if g >= 1:
                        if f == hook_b:
                            mid_B(g - 1)
                        elif f == hook_c:
                            mid_C(g - 1)
                mid_A_pre(g)
                flush_scores()
                mid_A(g)
                if g >= 2 and g - 2 != g_groups - 3:
                    pass2_finish(g - 2)
            # tail
            gl = g_groups - 1
            pass2_stretch(gl - 2, tail_split, (tail_split + NB) // 2)
            mid_B(gl)
            pass2_stretch(gl - 2, (tail_split + NB) // 2, NB)
            mid_C(gl)
            pass2_finish(gl - 2)
            for cb in range(LA):
                pass2_load(gl - 1, cb)
            pass2_start(gl - 1)
            pass2_stretch(gl - 1, 0, NB)
            pass2_finish(gl - 1)
            for cb in range(LA):
                pass2_load(gl, cb)
            pass2_start(gl)
            pass2_stretch(gl, 0, NB)
            pass2_finish(gl)

    nc.compile()
    return nc


def _prepare(x, batch, W1, b1, W2, b2, g_groups):
    """Host-side sharding/packing.  Returns (in_maps, gmax, spg, b2val)."""
    x = np.ascontiguousarray(np.asarray(x, dtype=np.float32))
    batch = np.asarray(batch).astype(np.int64)
    spg = SEG_PER_CORE // g_groups

    bounds = np.searchsorted(batch, np.arange(NSEG + 1))
    glens = bounds[spg:NSEG + 1:spg] - bounds[0:NSEG:spg]  # len per (core,group)
    gmax = max(512, _round_up(int(glens.max()), 512))
    C = gmax // 128
    F = gmax // 512

    xb = x.astype(NPBF16)
    w1b = np.ascontiguousarray(np.asarray(W1, np.float32).astype(NPBF16))
    w2b = np.ascontiguousarray(np.asarray(W2, np.float32).astype(NPBF16).reshape(H2, 1))
    b1f = np.ascontiguousarray(np.asarray(b1, np.float32).reshape(H2))
    b2val = float(np.asarray(b2, np.float32).reshape(-1)[0])

    in_maps = []
    for core in range(NCORES):
        x_nm = np.zeros((g_groups, 128, C, HID), NPBF16)
        x_fm = np.zeros((g_groups, 128, F, 4, 512), NPBF16)
        st = np.zeros((g_groups, 128, C, spg), np.float32)
        for g in range(g_groups):
            s0 = core * SEG_PER_CORE + g * spg
            n0, n1 = int(bounds[s0]), int(bounds[s0 + spg])
            L = n1 - n0
            xg = np.zeros((gmax, HID), NPBF16)
            xg[:L] = xb[n0:n1]
            # node-major: [p, c, hid], node = c*128 + p
            x_nm[g] = xg.reshape(C, 128, HID).transpose(1, 0, 2)
            # feature-major: [p, f, k, n], hid = k*128 + p, node = f*512 + n
            xT = np.ascontiguousarray(xg.T)  # [HID, gmax]
            x_fm[g] = xT.reshape(4, 128, F, 512).transpose(1, 2, 0, 3)
            oh = np.zeros((gmax, spg), np.float32)
            oh[np.arange(L), (batch[n0:n1] - s0).astype(np.int64)] = 1.0
            st[g] = oh.reshape(C, 128, spg).transpose(1, 0, 2)
        in_maps.append(
            {
                "x_nm": x_nm,
                "x_fm": x_fm,
                "st": st,
                "w1": w1b,
                "b1": b1f,
                "w2": w2b,
            }
        )
    return in_maps, gmax, spg, b2val


def _run(inputs, trace=False, **run_kwargs):
    in_maps, gmax, spg, b2val = _prepare(
        inputs["x"], inputs["batch"], inputs["W1"], inputs["b1"],
        inputs["W2"], inputs["b2"], G,
    )
    nc = _build_graph(G, gmax, spg, b2val)
    res = run_bass_kernel_spmd(
        nc, in_maps, core_ids=list(range(NCORES)), trace=trace, **run_kwargs
    )
    out = np.concatenate([r["out"] for r in res.results], axis=0)
    return out.astype(np.float32), res


def kernel(**inputs) -> np.ndarray:
    out, _ = _run(inputs, trace=False)
    return out
